# revision 6
# baseline (speedup 1.0000x reference)
# Trainium2 Bass kernel for nn_BinaryConv (binarized VGG-ish CNN, batch 512).
#
# Strategy: pure data parallel over 8 NeuronCores (64 images each), weights
# replicated. All activations are kept as a bf16 hi/lo pair (hi = bf16(x),
# lo = bf16(x - hi)) so every conv/fc runs as 2 bf16 matmuls accumulating in
# fp32 PSUM -> ~16-bit effective mantissa. The binarized (+-1) weights are
# exact in bf16. The network's logits are ~1e12 with min top-2 relative gap
# 5e-4, so bf16 alone flips argmaxes while hi/lo reproduces the fp32
# reference's (exactly one-hot) softmax bitwise.
#
# Per layer: conv = 9 shifted-window matmuls per output-row chunk (N<=512)
# accumulated in one PSUM bank; BN+bias folded into per-channel scale/bias
# applied by the scalar engine (Relu) straight out of PSUM; 2x2 maxpool on
# the vector engine on exact fp32 values before the hi/lo split. Layer 1
# uses host-side im2col with K=54 (27 taps x hi/lo) so one matmul per chunk.

import numpy as np
import ml_dtypes

import concourse.mybir as mybir
import concourse.tile as tile
from concourse import bacc
from concourse.bass_utils import run_bass_kernel_spmd

bf16 = ml_dtypes.bfloat16
F32 = mybir.dt.float32
BF16 = mybir.dt.bfloat16
Relu = mybir.ActivationFunctionType.Relu
Exp = mybir.ActivationFunctionType.Exp
Identity = mybir.ActivationFunctionType.Identity
MULT = mybir.AluOpType.mult
SUB = mybir.AluOpType.subtract
MAX = mybir.AluOpType.max
ADD = mybir.AluOpType.add

N_CORES = 8
B = 64          # images per core
SB = 16         # L1/L2 sub-batch
N_SB = 4
EPS = 1e-5

_NC_CACHE = {}
DEBUG_TAPS = False  # adds intermediate-tensor outputs for debugging


def _split_hi_lo(nc, pool, lo_dst, y32, hi_dst):
    # hi = bf16(relu-ish copy), lo = bf16(y32 - hi). y32 is already >= 0.
    nc.scalar.activation(hi_dst, y32, Relu)
    nc.vector.scalar_tensor_tensor(lo_dst, y32, 1.0, hi_dst, op0=MULT, op1=SUB)


def build_nc():
    if "nc" in _NC_CACHE:
        return _NC_CACHE["nc"]
    nc = bacc.Bacc(None, target_bir_lowering=False, debug=False)

    # ---------------- DRAM parameters ----------------
    xi = nc.declare_dram_parameter("xi", [N_SB, 54, 30 * 30 * SB], BF16, isOutput=False)
    w1 = nc.declare_dram_parameter("w1", [54, 128], BF16, isOutput=False)
    w2 = nc.declare_dram_parameter("w2", [128, 9, 128], BF16, isOutput=False)
    w3 = nc.declare_dram_parameter("w3", [128, 9, 256], BF16, isOutput=False)
    w4 = nc.declare_dram_parameter("w4", [2, 128, 9, 256], BF16, isOutput=False)
    w5 = nc.declare_dram_parameter("w5", [2, 128, 9, 512], BF16, isOutput=False)
    w6 = nc.declare_dram_parameter("w6", [4, 128, 9, 512], BF16, isOutput=False)
    fw1 = nc.declare_dram_parameter("fw1", [4, 128, 1024], BF16, isOutput=False)
    fw2 = nc.declare_dram_parameter("fw2", [8, 128, 1024], BF16, isOutput=False)
    fw3 = nc.declare_dram_parameter("fw3", [128, 8, 10], BF16, isOutput=False)
    # consts columns: 0:s1 1:t1 2:s2 3:t2 4-5:s3 6-7:t3 8-9:s4 10-11:t4
    # 12-15:s5 16-19:t5 20-23:s6 24-27:t6 28-35:fb1 36-43:fb2 44:fb3(rows0-9)
    consts = nc.declare_dram_parameter("consts", [128, 45], F32, isOutput=False)
    ident = nc.declare_dram_parameter("ident", [16, 16], F32, isOutput=False)
    out = nc.declare_dram_parameter("out", [B, 10], F32, isOutput=True)
    taps = {}
    if DEBUG_TAPS:
        for nm, shp in [("d_l1h", [128, 30, 30, SB]), ("d_l1l", [128, 30, 30, SB]),
                        ("d_p1h", [128, 14, 14, B]), ("d_p1l", [128, 14, 14, B]),
                        ("d_l3h", [128, 12, 12, B]), ("d_p2h", [128, 5, 5, B]),
                        ("d_l5h", [128, 3, 3, B]), ("d_fth", [128, B]),
                        ("d_z1h", [128, B]), ("d_z2h", [128, B]),
                        ("d_logits", [10, B])]:
            taps[nm] = nc.declare_dram_parameter(nm, shp, F32 if nm == "d_logits" else BF16,
                                                 isOutput=True)

    with tile.TileContext(nc) as tc:
        with tc.tile_pool(name="psp", bufs=8, space="PSUM") as psp, \
             tc.tile_pool(name="p0", bufs=1) as p0:
            # ---------------- persistent tiles ----------------
            w1s = p0.tile([54, 128], BF16)
            w2s = p0.tile([128, 9, 128], BF16)
            w3s = p0.tile([128, 9, 256], BF16)
            w4s = [p0.tile([128, 9, 256], BF16, name=f"w4s{i}") for i in range(2)]
            fw1s = [p0.tile([128, 1024], BF16, name=f"fw1s{i}") for i in range(4)]
            fw3s = p0.tile([128, 8, 10], BF16)
            cs = p0.tile([128, 45], F32)
            ids = p0.tile([16, 16], F32)
            p1h = p0.tile([128, 14, 14, B], BF16)
            p1l = p0.tile([128, 14, 14, B], BF16)
            p2h = [p0.tile([128, 5, 5, B], BF16, name=f"p2h{i}") for i in range(2)]
            p2l = [p0.tile([128, 5, 5, B], BF16, name=f"p2l{i}") for i in range(2)]
            l5h = [p0.tile([128, 3, 3, B], BF16, name=f"l5h{i}") for i in range(4)]
            l5l = [p0.tile([128, 3, 3, B], BF16, name=f"l5l{i}") for i in range(4)]
            fth = [p0.tile([128, B], BF16, name=f"fth{i}") for i in range(4)]
            ftl = [p0.tile([128, B], BF16, name=f"ftl{i}") for i in range(4)]
            z1h = [p0.tile([128, B], BF16, name=f"z1h{i}") for i in range(8)]
            z1l = [p0.tile([128, B], BF16, name=f"z1l{i}") for i in range(8)]
            z2h = [p0.tile([128, B], BF16, name=f"z2h{i}") for i in range(8)]
            z2l = [p0.tile([128, B], BF16, name=f"z2l{i}") for i in range(8)]

            nc.sync.dma_start(out=w1s[:], in_=w1[:])
            nc.sync.dma_start(out=w2s[:], in_=w2[:])
            nc.sync.dma_start(out=w3s[:], in_=w3[:])
            for i in range(2):
                nc.sync.dma_start(out=w4s[i][:], in_=w4[i])
            for i in range(4):
                nc.sync.dma_start(out=fw1s[i][:], in_=fw1[i])
            nc.sync.dma_start(out=fw3s[:], in_=fw3[:])
            nc.sync.dma_start(out=cs[:], in_=consts[:])
            nc.sync.dma_start(out=ids[:], in_=ident[:])

            def col(j):
                return cs[:, j:j + 1]

            # =============== phase A: L1, L2, pool1 (per sub-batch) ===============
            with tc.tile_pool(name="pA", bufs=1) as pA:
                for sb in range(N_SB):
                    l1h = pA.tile([128, 30, 30, SB], BF16, tag="l1h")
                    l1l = pA.tile([128, 30, 30, SB], BF16, tag="l1l")
                    # ---- L1: im2col K=54, one matmul per output row ----
                    for r in range(30):
                        ic = pA.tile([54, 30, SB], BF16, tag="ic", bufs=3)
                        nc.sync.dma_start(
                            out=ic[:], in_=xi[sb, :, r * 30 * SB:(r + 1) * 30 * SB])
                        ps = psp.tile([128, 30, SB], F32, tag="ps")
                        nc.tensor.matmul(ps[:], w1s[:], ic[:], start=True, stop=True)
                        y32 = pA.tile([128, 30, SB], F32, tag="y32", bufs=3)
                        nc.scalar.activation(y32[:], ps[:], Relu, bias=col(1), scale=col(0))
                        nc.scalar.activation(l1h[:, r], ps[:], Relu, bias=col(1), scale=col(0))
                        nc.vector.scalar_tensor_tensor(
                            l1l[:, r], y32[:], 1.0, l1h[:, r], op0=MULT, op1=SUB)
                    if DEBUG_TAPS and sb == 0:
                        nc.sync.dma_start(out=taps["d_l1h"][:], in_=l1h[:])
                        nc.sync.dma_start(out=taps["d_l1l"][:], in_=l1l[:])
                    # ---- L2 + pool1 ----
                    for p in range(14):
                        rows = []
                        for rr in range(2):
                            r = 2 * p + rr
                            ps = psp.tile([128, 28, SB], F32, tag="ps")
                            first = True
                            for part in (l1h, l1l):
                                for dh in range(3):
                                    for dw in range(3):
                                        nc.tensor.matmul(
                                            ps[:], w2s[:, dh * 3 + dw, :],
                                            part[:, r + dh, dw:dw + 28, :],
                                            start=first, stop=(part is l1l and dh == 2 and dw == 2))
                                        first = False
                            y32 = pA.tile([128, 28, SB], F32, tag="y32", bufs=3,
                                          name=f"y2_{sb}_{p}_{rr}")
                            nc.scalar.activation(y32[:], ps[:], Relu, bias=col(3), scale=col(2))
                            rows.append(y32)
                        rm = pA.tile([128, 28, SB], F32, tag="rm", bufs=2)
                        nc.vector.tensor_tensor(rm[:], rows[0][:], rows[1][:], op=MAX)
                        rmv = rm[:].rearrange("p (w two) b -> p w two b", two=2)
                        pw = pA.tile([128, 14, SB], F32, tag="pw", bufs=2)
                        nc.vector.tensor_tensor(pw[:], rmv[:, :, 0, :], rmv[:, :, 1, :], op=MAX)
                        bsl = slice(sb * SB, (sb + 1) * SB)
                        nc.scalar.activation(p1h[:, p, :, bsl], pw[:], Relu)
                        nc.vector.scalar_tensor_tensor(
                            p1l[:, p, :, bsl], pw[:], 1.0, p1h[:, p, :, bsl],
                            op0=MULT, op1=SUB)

            if DEBUG_TAPS:
                nc.sync.dma_start(out=taps["d_p1h"][:], in_=p1h[:])
                nc.sync.dma_start(out=taps["d_p1l"][:], in_=p1l[:])
            # =============== phase B: L3, L4, pool2 (full batch) ===============
            with tc.tile_pool(name="pB", bufs=1) as pB:
                l3h = [pB.tile([128, 12, 12, B], BF16, name=f"l3h{i}") for i in range(2)]
                l3l = [pB.tile([128, 12, 12, B], BF16, name=f"l3l{i}") for i in range(2)]
                # ---- L3 ----
                for cog in range(2):
                    wsl = slice(cog * 128, (cog + 1) * 128)
                    for r in range(12):
                        for bh in range(2):
                            bsl = slice(bh * 32, (bh + 1) * 32)
                            ps = psp.tile([128, 12, 32], F32, tag="ps")
                            first = True
                            for part in (p1h, p1l):
                                for dh in range(3):
                                    for dw in range(3):
                                        nc.tensor.matmul(
                                            ps[:], w3s[:, dh * 3 + dw, wsl],
                                            part[:, r + dh, dw:dw + 12, bsl],
                                            start=first,
                                            stop=(part is p1l and dh == 2 and dw == 2))
                                        first = False
                            y32 = pB.tile([128, 12, 32], F32, tag="y32b", bufs=4,
                                          name=f"y3_{cog}_{r}_{bh}")
                            nc.scalar.activation(y32[:], ps[:], Relu,
                                                 bias=col(6 + cog), scale=col(4 + cog))
                            nc.scalar.activation(l3h[cog][:, r, :, bsl], ps[:], Relu,
                                                 bias=col(6 + cog), scale=col(4 + cog))
                            nc.vector.scalar_tensor_tensor(
                                l3l[cog][:, r, :, bsl], y32[:], 1.0,
                                l3h[cog][:, r, :, bsl], op0=MULT, op1=SUB)
                if DEBUG_TAPS:
                    nc.sync.dma_start(out=taps["d_l3h"][:], in_=l3h[0][:])
                # ---- L4 + pool2 ----
                for cog in range(2):
                    wsl = slice(cog * 128, (cog + 1) * 128)
                    for bh in range(2):
                        bsl = slice(bh * 32, (bh + 1) * 32)
                        for p in range(5):
                            rows = []
                            for rr in range(2):
                                r = 2 * p + rr
                                ps = psp.tile([128, 10, 32], F32, tag="ps")
                                first = True
                                for cb in range(2):
                                    for part in (l3h, l3l):
                                        for dh in range(3):
                                            for dw in range(3):
                                                nc.tensor.matmul(
                                                    ps[:], w4s[cb][:, dh * 3 + dw, wsl],
                                                    part[cb][:, r + dh, dw:dw + 10, bsl],
                                                    start=first,
                                                    stop=(cb == 1 and part is l3l
                                                          and dh == 2 and dw == 2))
                                                first = False
                                y32 = pB.tile([128, 10, 32], F32, tag="y32b", bufs=4,
                                              name=f"y4_{cog}_{bh}_{p}_{rr}")
                                nc.scalar.activation(y32[:], ps[:], Relu,
                                                     bias=col(10 + cog), scale=col(8 + cog))
                                rows.append(y32)
                            rm = pB.tile([128, 10, 32], F32, tag="rm4", bufs=2)
                            nc.vector.tensor_tensor(rm[:], rows[0][:], rows[1][:], op=MAX)
                            rmv = rm[:].rearrange("p (w two) b -> p w two b", two=2)
                            pw = pB.tile([128, 5, 32], F32, tag="pw4", bufs=2)
                            nc.vector.tensor_tensor(pw[:], rmv[:, :, 0, :], rmv[:, :, 1, :],
                                                    op=MAX)
                            nc.scalar.activation(p2h[cog][:, p, :, bsl], pw[:], Relu)
                            nc.vector.scalar_tensor_tensor(
                                p2l[cog][:, p, :, bsl], pw[:], 1.0,
                                p2h[cog][:, p, :, bsl], op0=MULT, op1=SUB)

            # =============== phase C: L5, L6, FC, softmax ===============
            with tc.tile_pool(name="pC", bufs=1) as pC:
                w5s = [pC.tile([128, 9, 512], BF16, name=f"w5s{i}") for i in range(2)]
                w6s = [pC.tile([128, 9, 512], BF16, name=f"w6s{i}") for i in range(4)]
                fw2s = [pC.tile([128, 1024], BF16, name=f"fw2s{i}") for i in range(8)]
                for i in range(2):
                    nc.sync.dma_start(out=w5s[i][:], in_=w5[i])
                for i in range(4):
                    nc.sync.dma_start(out=w6s[i][:], in_=w6[i])
                for i in range(8):
                    nc.sync.dma_start(out=fw2s[i][:], in_=fw2[i])

                if DEBUG_TAPS:
                    nc.sync.dma_start(out=taps["d_p2h"][:], in_=p2h[0][:])
                # ---- L5 ----
                for cog in range(4):
                    wsl = slice(cog * 128, (cog + 1) * 128)
                    for (h0, nh) in ((0, 2), (2, 1)):
                        ps = psp.tile([128, nh, 3, B], F32, tag="ps")
                        first = True
                        for cb in range(2):
                            for part in (p2h, p2l):
                                for dh in range(3):
                                    for dw in range(3):
                                        nc.tensor.matmul(
                                            ps[:], w5s[cb][:, dh * 3 + dw, wsl],
                                            part[cb][:, h0 + dh:h0 + dh + nh, dw:dw + 3, :],
                                            start=first,
                                            stop=(cb == 1 and part is p2l
                                                  and dh == 2 and dw == 2))
                                        first = False
                        y32 = pC.tile([128, nh, 3, B], F32, tag="y32c", bufs=3,
                                      name=f"y5_{cog}_{h0}")
                        nc.scalar.activation(y32[:], ps[:], Relu,
                                             bias=col(16 + cog), scale=col(12 + cog))
                        nc.scalar.activation(l5h[cog][:, h0:h0 + nh], ps[:], Relu,
                                             bias=col(16 + cog), scale=col(12 + cog))
                        nc.vector.scalar_tensor_tensor(
                            l5l[cog][:, h0:h0 + nh], y32[:], 1.0,
                            l5h[cog][:, h0:h0 + nh], op0=MULT, op1=SUB)

                if DEBUG_TAPS:
                    nc.sync.dma_start(out=taps["d_l5h"][:], in_=l5h[0][:])
                # ---- L6 (3x3 conv on 3x3 input == dense over (ci, s)) ----
                for cog in range(4):
                    wsl = slice(cog * 128, (cog + 1) * 128)
                    ps = psp.tile([128, B], F32, tag="ps")
                    first = True
                    for cb in range(4):
                        for part in (l5h, l5l):
                            pv = part[cb][:].rearrange("p h w b -> p (h w) b")
                            for s in range(9):
                                nc.tensor.matmul(
                                    ps[:], w6s[cb][:, s, wsl], pv[:, s, :],
                                    start=first,
                                    stop=(cb == 3 and part is l5l and s == 8))
                                first = False
                    y32 = pC.tile([128, B], F32, tag="y32c", bufs=3, name=f"y6_{cog}")
                    nc.scalar.activation(y32[:], ps[:], Relu,
                                         bias=col(24 + cog), scale=col(20 + cog))
                    nc.scalar.activation(fth[cog][:], ps[:], Relu,
                                         bias=col(24 + cog), scale=col(20 + cog))
                    nc.vector.scalar_tensor_tensor(
                        ftl[cog][:], y32[:], 1.0, fth[cog][:], op0=MULT, op1=SUB)

                if DEBUG_TAPS:
                    nc.sync.dma_start(out=taps["d_fth"][:], in_=fth[0][:])
                # ---- FC1 ----
                for cog in range(8):
                    wsl = slice(cog * 128, (cog + 1) * 128)
                    ps = psp.tile([128, B], F32, tag="ps")
                    first = True
                    for kb in range(4):
                        for part in (fth, ftl):
                            nc.tensor.matmul(ps[:], fw1s[kb][:, wsl], part[kb][:],
                                             start=first,
                                             stop=(kb == 3 and part is ftl))
                            first = False
                    y32 = pC.tile([128, B], F32, tag="y32c", bufs=3, name=f"yf1_{cog}")
                    nc.scalar.activation(y32[:], ps[:], Relu, bias=col(28 + cog), scale=1.0)
                    nc.scalar.activation(z1h[cog][:], ps[:], Relu, bias=col(28 + cog), scale=1.0)
                    nc.vector.scalar_tensor_tensor(
                        z1l[cog][:], y32[:], 1.0, z1h[cog][:], op0=MULT, op1=SUB)

                if DEBUG_TAPS:
                    nc.sync.dma_start(out=taps["d_z1h"][:], in_=z1h[0][:])
                # ---- FC2 ----
                for cog in range(8):
                    wsl = slice(cog * 128, (cog + 1) * 128)
                    ps = psp.tile([128, B], F32, tag="ps")
                    first = True
                    for kb in range(8):
                        for part in (z1h, z1l):
                            nc.tensor.matmul(ps[:], fw2s[kb][:, wsl], part[kb][:],
                                             start=first,
                                             stop=(kb == 7 and part is z1l))
                            first = False
                    y32 = pC.tile([128, B], F32, tag="y32c", bufs=3, name=f"yf2_{cog}")
                    nc.scalar.activation(y32[:], ps[:], Relu, bias=col(36 + cog), scale=1.0)
                    nc.scalar.activation(z2h[cog][:], ps[:], Relu, bias=col(36 + cog), scale=1.0)
                    nc.vector.scalar_tensor_tensor(
                        z2l[cog][:], y32[:], 1.0, z2h[cog][:], op0=MULT, op1=SUB)

                if DEBUG_TAPS:
                    nc.sync.dma_start(out=taps["d_z2h"][:], in_=z2h[0][:])
                # ---- FC3 + softmax ----
                ps = psp.tile([10, B], F32, tag="ps")
                first = True
                for kb in range(8):
                    for part in (z2h, z2l):
                        nc.tensor.matmul(ps[:], fw3s[:, kb, :], part[kb][:],
                                         start=first, stop=(kb == 7 and part is z2l))
                        first = False
                logits = pC.tile([10, B], F32)
                nc.scalar.activation(logits[:], ps[:], Identity, bias=cs[0:10, 44:45], scale=1.0)
                if DEBUG_TAPS:
                    nc.sync.dma_start(out=taps["d_logits"][:], in_=logits[:])
                pst = psp.tile([B, 10], F32, tag="ps")
                nc.tensor.transpose(pst[:], logits[:], ids[0:10, 0:10])
                zt = pC.tile([B, 10], F32)
                nc.vector.tensor_copy(zt[:], pst[:])
                nm = pC.tile([B, 1], F32)
                nc.vector.tensor_reduce(out=nm[:], in_=zt[:], op=MAX,
                                        axis=mybir.AxisListType.X, negate=True)
                ex = pC.tile([B, 10], F32)
                nc.scalar.activation(ex[:], zt[:], Exp, bias=nm[:], scale=1.0)
                sm = pC.tile([B, 1], F32)
                nc.vector.tensor_reduce(out=sm[:], in_=ex[:], op=ADD,
                                        axis=mybir.AxisListType.X)
                rc = pC.tile([B, 1], F32)
                nc.vector.reciprocal(rc[:], sm[:])
                so = pC.tile([B, 10], F32)
                nc.vector.tensor_scalar_mul(so[:], ex[:], rc[:])
                nc.sync.dma_start(out=out[:], in_=so[:])

    nc.compile()
    _NC_CACHE["nc"] = nc
    return nc


# ---------------- host-side data prep ----------------

def _fold_bn(b, g, be, m, v):
    inv = (g / np.sqrt(v + EPS)).astype(np.float32)
    return inv, ((b - m) * inv + be).astype(np.float32)


def _conv_w(w):
    # [co, ci, kh, kw] +-1 -> [ci, kh*3+kw, co] bf16 (split over 128-blocks of ci outside)
    return np.ascontiguousarray(np.sign(w).transpose(1, 2, 3, 0).reshape(
        w.shape[1], 9, w.shape[0])).astype(bf16)


def _prep_shared(inputs):
    d = {}
    d["w1"] = np.vstack([_conv_w(inputs["w1"]).reshape(27, 128)] * 2)
    d["w2"] = _conv_w(inputs["w2"])
    d["w3"] = _conv_w(inputs["w3"])
    d["w4"] = np.ascontiguousarray(_conv_w(inputs["w4"]).reshape(2, 128, 9, 256))
    d["w5"] = np.ascontiguousarray(_conv_w(inputs["w5"]).reshape(2, 128, 9, 512))
    d["w6"] = np.ascontiguousarray(_conv_w(inputs["w6"]).reshape(4, 128, 9, 512))
    for nm, k in (("fw1", 4), ("fw2", 8)):
        w = np.sign(inputs[nm]).T.astype(bf16)  # [K, co]
        d[nm] = np.ascontiguousarray(w.reshape(k, 128, w.shape[1]))
    w = np.sign(inputs["fw3"]).T.astype(bf16)  # [1024, 10]
    d["fw3"] = np.ascontiguousarray(w.reshape(8, 128, 10).transpose(1, 0, 2))
    consts = np.zeros((128, 45), np.float32)
    coff = [(1, 0, 1), (2, 2, 3), (3, 4, 6), (4, 8, 10), (5, 12, 16), (6, 20, 24)]
    for li, so, to in coff:
        s, t = _fold_bn(inputs[f"b{li}"], inputs[f"g{li}"], inputs[f"be{li}"],
                        inputs[f"m{li}"], inputs[f"v{li}"])
        nb = len(s) // 128
        for j in range(nb):
            consts[:, so + j] = s[j * 128:(j + 1) * 128]
            consts[:, to + j] = t[j * 128:(j + 1) * 128]
    for j in range(8):
        consts[:, 28 + j] = inputs["fb1"][j * 128:(j + 1) * 128]
        consts[:, 36 + j] = inputs["fb2"][j * 128:(j + 1) * 128]
    consts[0:10, 44] = inputs["fb3"]
    d["consts"] = consts
    d["ident"] = np.eye(16, dtype=np.float32)
    return d


def _prep_x(xc):
    # xc [B, 3, 32, 32] f32 -> im2col [N_SB, 54, 30*30*SB] bf16 (hi rows 0-26, lo 27-53)
    x32 = xc.astype(np.float32)
    hi = x32.astype(bf16)
    lo = (x32 - hi.astype(np.float32)).astype(bf16)
    parts = []
    for p in (hi, lo):
        win = np.lib.stride_tricks.sliding_window_view(p, (3, 3), axis=(2, 3))
        # win [B, ci, r, w, dh, dw] -> [ci, dh, dw, r, w, B]
        arr = win.transpose(1, 4, 5, 2, 3, 0).reshape(27, 30, 30, B)
        parts.append(arr)
    full = np.concatenate(parts, axis=0)  # [54, 30, 30, B]
    full = full.reshape(54, 30, 30, N_SB, SB).transpose(3, 0, 1, 2, 4)
    return np.ascontiguousarray(full).reshape(N_SB, 54, 30 * 30 * SB)


def make_in_maps(inputs):
    shared = _prep_shared(inputs)
    x = np.asarray(inputs["x"])
    in_maps = []
    for c in range(N_CORES):
        m = dict(shared)
        m["xi"] = _prep_x(x[c * B:(c + 1) * B])
        in_maps.append(m)
    return in_maps


def kernel(**inputs):
    nc = build_nc()
    in_maps = make_in_maps(inputs)
    res = run_bass_kernel_spmd(nc, in_maps, list(range(N_CORES)))
    return np.concatenate([res.results[c]["out"] for c in range(N_CORES)], axis=0)
